# revision 17
# baseline (speedup 1.0000x reference)
"""Trainium2 Bass kernel for ShallowLSTM seq2seq (encoder-decoder LSTM).

Sharding: pure data parallel — batch 256 split as 32 per core across 8
NeuronCores; all weights replicated.

Per-core layout ("replicated batch" scheme):
  - Local batch is 32 < 128 PE rows. Matmul stationaries are h^T tiles
    [128 hdim-chunk, 128] whose 128 M-columns are the 32-batch replicated
    4x, so every matmul fills all 128 PSUM partitions with 4 identical
    copies of the gates. Pointwise then runs at full 128-lane utilization
    and a single [128,128] PE-transpose per hidden chunk regenerates the
    next step's (replicated) h^T stationary directly.
  - Gates PSUM tile is [128, 2048], chunk-permuted so PSUM bank n holds
    [i_n, f_n, o_n, g_n] for hidden units 128n:128(n+1) (contiguous
    sigmoid span [0:384], tanh span [384:512]).
  - Matmuls run in bf16 (fp32 PSUM accumulation; fp32 pointwise state) —
    bf16 operands take the separate-LDWEIGHTS path instead of the slow
    self-loading fp32/fp32r path, and measured end-to-end error stays
    ~3.5e-3 because the LSTM gate recursion is contractive.
  - All weight transposes/permutations + x transpose/replication are done
    host-side in numpy; tensors arrive in DRAM pre-laid-out.
"""

import numpy as np
import ml_dtypes

import concourse.bass as bass
import concourse.bacc as bacc
import concourse.tile as tile
from concourse import mybir
from concourse.bass_utils import run_bass_kernel_spmd

B, T_IN, F, H = 256, 168, 128, 512
T_OUT = 48
N_CORES = 8
BL = B // N_CORES  # 32 local batch
G4 = 4 * H  # 2048
NB = 4  # PSUM gate banks / hidden chunks
SLAB = 8  # encoder x slab, steps per DMA

AF = mybir.ActivationFunctionType
F32 = mybir.dt.float32
F32R = mybir.dt.float32r
BF16 = mybir.dt.bfloat16


def _r(ap):
    return ap.bitcast(F32R)


def build_nc(t_enc=T_IN, t_dec=T_OUT):
    nc = bacc.Bacc("TRN2", target_bir_lowering=False)

    xrep_d = nc.declare_dram_parameter("xrep", [F, t_enc * 128], BF16, isOutput=False)
    whh_e_d = nc.declare_dram_parameter("whhT_enc", [H, G4], BF16, isOutput=False)
    wih_e_d = nc.declare_dram_parameter("wihT_enc", [F, G4], BF16, isOutput=False)
    b_e_d = nc.declare_dram_parameter("b_enc", [1, G4], BF16, isOutput=False)
    whh_d_d = nc.declare_dram_parameter("whhT_dec", [H, G4], BF16, isOutput=False)
    wih_d_d = nc.declare_dram_parameter("wihT_dec", [F, G4], BF16, isOutput=False)
    b_d_d = nc.declare_dram_parameter("b_dec", [1, G4], BF16, isOutput=False)
    wlin_d = nc.declare_dram_parameter("wlinT", [H, F], BF16, isOutput=False)
    ident_d = nc.declare_dram_parameter("ident", [128, 128], F32, isOutput=False)
    identb_d = nc.declare_dram_parameter("identb", [128, 128], BF16, isOutput=False)
    out_d = nc.declare_dram_parameter("out", [BL, t_dec * F], F32, isOutput=True)

    with tile.TileContext(nc) as tc:
        with (
            tc.tile_pool(name="weights", bufs=1) as wpool,
            tc.tile_pool(name="state", bufs=1) as spool,
            tc.tile_pool(name="xslab", bufs=3) as xpool,
            tc.tile_pool(name="sig", bufs=3) as sigpool,
            tc.tile_pool(name="tmp", bufs=4) as tmppool,
            tc.tile_pool(name="hbuf", bufs=3) as hpool,
            tc.tile_pool(name="gps", bufs=4, space=bass.MemorySpace.PSUM) as gpool,
            tc.tile_pool(name="htps", bufs=2, space=bass.MemorySpace.PSUM) as htpool,
            tc.tile_pool(name="wlps", bufs=1, space=bass.MemorySpace.PSUM) as wlpool,
        ):
            # ---- weights into SBUF ----
            whh_e = [wpool.tile([128, G4], BF16, tag=f"whh_e{j}", name=f"whh_e{j}") for j in range(4)]
            whh_dc = [wpool.tile([128, G4], BF16, tag=f"whh_d{j}", name=f"whh_d{j}") for j in range(4)]
            for j in range(4):
                nc.sync.dma_start(whh_e[j][:], whh_e_d[128 * j : 128 * (j + 1), :])
                nc.sync.dma_start(whh_dc[j][:], whh_d_d[128 * j : 128 * (j + 1), :])
            wih_e = wpool.tile([F, G4], BF16, tag="wih_e")
            wih_dc = wpool.tile([F, G4], BF16, tag="wih_d")
            nc.sync.dma_start(wih_e[:], wih_e_d[:])
            nc.sync.dma_start(wih_dc[:], wih_d_d[:])
            b_e = wpool.tile([1, G4], BF16, tag="b_e")
            b_dc = wpool.tile([1, G4], BF16, tag="b_d")
            nc.sync.dma_start(b_e[:], b_e_d[:])
            nc.sync.dma_start(b_dc[:], b_d_d[:])
            wlin = [wpool.tile([128, F], BF16, tag=f"wlin{j}", name=f"wlin{j}") for j in range(4)]
            for j in range(4):
                nc.sync.dma_start(wlin[j][:], wlin_d[128 * j : 128 * (j + 1), :])
            ident = wpool.tile([128, 128], F32, tag="ident")
            nc.sync.dma_start(ident[:], ident_d[:])
            identb = wpool.tile([128, 128], BF16, tag="identb")
            nc.sync.dma_start(identb[:], identb_d[:])
            ones = wpool.tile([1, 128], BF16, tag="ones")
            nc.vector.memset(ones[:], 1.0)

            # ---- state ----
            hT = [spool.tile([128, 128], BF16, tag=f"hT{j}", name=f"hT{j}") for j in range(4)]
            c_st = [spool.tile([128, 128], F32, tag=f"c{j}", name=f"c{j}") for j in range(4)]
            for j in range(4):
                nc.vector.memset(hT[j][:], 0.0)
                nc.vector.memset(c_st[j][:], 0.0)
            out_store = spool.tile([128, t_dec * BL], F32, tag="out_store")

            def lstm_step(x_lhsT, whh, wih, b_t, first):
                """One LSTM cell step. x_lhsT: [F,128] fp32 stationary
                (batch-replicated x_t^T). Updates hT[j], c_st[j] in place."""
                gps = []
                for n in range(NB):
                    g_ps = gpool.tile([128, 512], F32, tag="g_ps")
                    ncol = slice(512 * n, 512 * (n + 1))
                    nc.tensor.matmul(
                        g_ps[:], ones[:], b_t[:, ncol], start=True, stop=False
                    )
                    nc.tensor.matmul(
                        g_ps[:], x_lhsT, wih[:, ncol], start=False, stop=False
                    )
                    for j in range(4):
                        nc.tensor.matmul(
                            g_ps[:],
                            hT[j][:],
                            whh[j][:, ncol],
                            start=False,
                            stop=(j == 3),
                        )
                    gps.append(g_ps)
                for n in range(NB):
                    g_ps = gps[n]
                    # bank layout: [i(0:128), f(128:256), o(256:384), g(384:512)]
                    sig = sigpool.tile([128, 384], F32, tag="sig")
                    nc.scalar.activation(sig[:], g_ps[:, 0:384], AF.Sigmoid)
                    tg = tmppool.tile([128, 128], F32, tag="tg")
                    nc.scalar.activation(tg[:], g_ps[:, 384:512], AF.Tanh)
                    t1 = tmppool.tile([128, 128], F32, tag="t1")
                    nc.vector.tensor_mul(t1[:], sig[:, 128:256], c_st[n][:])
                    t2 = tmppool.tile([128, 128], F32, tag="t2")
                    nc.vector.tensor_mul(t2[:], sig[:, 0:128], tg[:])
                    nc.vector.tensor_add(c_st[n][:], t1[:], t2[:])
                    tc_t = tmppool.tile([128, 128], F32, tag="tc")
                    nc.scalar.activation(tc_t[:], c_st[n][:], AF.Tanh)
                    h_t = hpool.tile([128, 128], BF16, tag="h")
                    nc.vector.tensor_mul(h_t[:], sig[:, 256:384], tc_t[:])
                    ht_ps = htpool.tile([128, 128], BF16, tag="ht_ps")
                    nc.tensor.transpose(ht_ps[:], h_t[:], identb[:])
                    nc.vector.tensor_copy(hT[n][:], ht_ps[:])

            # ---- encoder ----
            xsl = None
            for t in range(t_enc):
                if t % SLAB == 0:
                    nsteps = min(SLAB, t_enc - t)
                    xsl = xpool.tile([F, SLAB * 128], BF16, tag="xsl")
                    nc.sync.dma_start(
                        xsl[:, : nsteps * 128],
                        xrep_d[:, t * 128 : (t + nsteps) * 128],
                    )
                k = t % SLAB
                lstm_step(
                    xsl[:, k * 128 : (k + 1) * 128], whh_e, wih_e, b_e, first=(t == 0)
                )

            # ---- decoder ----
            x_last = spool.tile([F, 128], BF16, tag="x_last")
            nc.sync.dma_start(x_last[:], xrep_d[:, (t_enc - 1) * 128 : t_enc * 128])
            dec_in = x_last
            for t in range(t_dec):
                lstm_step(dec_in[:], whh_dc, wih_dc, b_dc, first=False)
                wl_ps = wlpool.tile([128, 128], F32, tag="wl_ps")
                for j in range(4):
                    nc.tensor.matmul(
                        wl_ps[:],
                        wlin[j][:],
                        hT[j][:],
                        start=(j == 0),
                        stop=(j == 3),
                    )
                outT = hpool.tile([128, 128], BF16, tag="outT")
                nc.vector.tensor_copy(outT[:], wl_ps[:])
                nc.scalar.activation(
                    out_store[:, t * BL : (t + 1) * BL], wl_ps[:, 0:BL], AF.Copy
                )
                dec_in = outT

            # ---- transpose outputs to batch-major and store ----
            out_final = spool.tile([BL, t_dec * F], F32, tag="out_final")
            for t in range(t_dec):
                tr_ps = wlpool.tile([BL, 128], F32, tag="tr_ps")
                nc.tensor.transpose(
                    tr_ps[:], out_store[:, t * BL : (t + 1) * BL], ident[:]
                )
                nc.scalar.activation(
                    out_final[:, t * F : (t + 1) * F], tr_ps[:], AF.Copy
                )
            nc.sync.dma_start(out_d[:], out_final[:])

    nc.compile()
    return nc


def _prep_host(inputs, t_enc=T_IN, t_dec=T_OUT):
    """Host-side transposes/permutations. Returns per-core in_maps."""
    x = np.asarray(inputs["x"], np.float32)
    bsz = x.shape[0]
    bl = bsz // N_CORES

    # chunk-permutation of the 2048 gate rows:
    # bank n holds [i_n, f_n, o_n, g_n] for hidden units 128n:128(n+1)
    # torch gate order in weights: i(0:512) f(512:1024) g(1024:1536) o(1536:2048)
    perm = np.concatenate(
        [
            np.concatenate(
                [
                    np.arange(128 * n, 128 * (n + 1)),  # i
                    512 + np.arange(128 * n, 128 * (n + 1)),  # f
                    1536 + np.arange(128 * n, 128 * (n + 1)),  # o
                    1024 + np.arange(128 * n, 128 * (n + 1)),  # g
                ]
            )
            for n in range(4)
        ]
    )

    def prep_w(wih, whh, b):
        wihT = np.ascontiguousarray(np.asarray(wih, np.float32)[perm].T).astype(ml_dtypes.bfloat16)
        whhT = np.ascontiguousarray(np.asarray(whh, np.float32)[perm].T).astype(ml_dtypes.bfloat16)
        bp = np.ascontiguousarray(np.asarray(b, np.float32)[perm][None, :]).astype(ml_dtypes.bfloat16)
        return wihT, whhT, bp

    wihT_e, whhT_e, b_e = prep_w(inputs["enc_Wih"], inputs["enc_Whh"], inputs["enc_b"])
    wihT_d, whhT_d, b_d = prep_w(inputs["dec_Wih"], inputs["dec_Whh"], inputs["dec_b"])
    wlinT = np.ascontiguousarray(np.asarray(inputs["W_lin"], np.float32).T).astype(ml_dtypes.bfloat16)
    ident = np.eye(128, dtype=np.float32)

    in_maps = []
    for c in range(N_CORES):
        xc = x[c * bl : (c + 1) * bl, :t_enc, :]  # [bl, t_enc, F]
        xt = xc.transpose(2, 1, 0)  # [F, t_enc, bl]
        xrep = np.broadcast_to(
            xt[:, :, None, :], (F, t_enc, 128 // bl, bl)
        ).reshape(F, t_enc * 128)
        in_maps.append(
            {
                "xrep": np.ascontiguousarray(xrep).astype(ml_dtypes.bfloat16),
                "whhT_enc": whhT_e,
                "wihT_enc": wihT_e,
                "b_enc": b_e,
                "whhT_dec": whhT_d,
                "wihT_dec": wihT_d,
                "b_dec": b_d,
                "wlinT": wlinT,
                "ident": ident,
                "identb": ident.astype(ml_dtypes.bfloat16),
            }
        )
    return in_maps


_NC_CACHE = {}


def _get_nc(t_enc, t_dec):
    key = (t_enc, t_dec)
    if key not in _NC_CACHE:
        _NC_CACHE[key] = build_nc(t_enc, t_dec)
    return _NC_CACHE[key]


def kernel(x, enc_Wih, enc_Whh, enc_b, dec_Wih, dec_Whh, dec_b, W_lin, t_out):
    t_out = int(t_out)
    inputs = dict(
        x=x, enc_Wih=enc_Wih, enc_Whh=enc_Whh, enc_b=enc_b,
        dec_Wih=dec_Wih, dec_Whh=dec_Whh, dec_b=dec_b, W_lin=W_lin,
    )
    t_enc = int(np.asarray(x).shape[1])
    nc = _get_nc(t_enc, t_out)
    in_maps = _prep_host(inputs, t_enc, t_out)
    res = run_bass_kernel_spmd(nc, in_maps, list(range(N_CORES)))
    bl = np.asarray(x).shape[0] // N_CORES
    outs = [
        res.results[c]["out"].reshape(bl, t_out, F) for c in range(N_CORES)
    ]
    return np.concatenate(outs, axis=0).astype(np.float32)


# revision 18
# speedup vs baseline: 1.1770x; 1.1770x over previous
"""Trainium2 Bass kernel for ShallowLSTM seq2seq (encoder-decoder LSTM).

Sharding: pure data parallel — batch 256 split as 32 per core across 8
NeuronCores; all weights replicated.

Per-core layout ("replicated batch" scheme):
  - Local batch is 32 < 128 PE rows. Matmul stationaries are h^T tiles
    [128 hdim-chunk, 128] whose 128 M-columns are the 32-batch replicated
    4x, so every matmul fills all 128 PSUM partitions with 4 identical
    copies of the gates. Pointwise then runs at full 128-lane utilization
    and a single [128,128] PE-transpose per hidden chunk regenerates the
    next step's (replicated) h^T stationary directly.
  - Gates PSUM tile is [128, 2048], chunk-permuted so PSUM bank n holds
    [i_n, f_n, o_n, g_n] for hidden units 128n:128(n+1) (contiguous
    sigmoid span [0:384], tanh span [384:512]).
  - Matmuls run in bf16 (fp32 PSUM accumulation; fp32 pointwise state) —
    bf16 operands take the separate-LDWEIGHTS path instead of the slow
    self-loading fp32/fp32r path, and measured end-to-end error stays
    ~3.5e-3 because the LSTM gate recursion is contractive.
  - All weight transposes/permutations + x transpose/replication are done
    host-side in numpy; tensors arrive in DRAM pre-laid-out.
"""

import numpy as np
import ml_dtypes

import concourse.bass as bass
import concourse.bacc as bacc
import concourse.tile as tile
from concourse import mybir
from concourse.bass_utils import run_bass_kernel_spmd

B, T_IN, F, H = 256, 168, 128, 512
T_OUT = 48
N_CORES = 8
BL = B // N_CORES  # 32 local batch
G4 = 4 * H  # 2048
NB = 4  # PSUM gate banks / hidden chunks
SLAB = 8  # encoder x slab, steps per DMA

AF = mybir.ActivationFunctionType
F32 = mybir.dt.float32
F32R = mybir.dt.float32r
BF16 = mybir.dt.bfloat16


def _r(ap):
    return ap.bitcast(F32R)


def build_nc(t_enc=T_IN, t_dec=T_OUT):
    nc = bacc.Bacc("TRN2", target_bir_lowering=False)

    xrep_d = nc.declare_dram_parameter("xrep", [F, t_enc * 128], BF16, isOutput=False)
    whh_e_d = nc.declare_dram_parameter("whhT_enc", [H, G4], BF16, isOutput=False)
    wih_e_d = nc.declare_dram_parameter("wihT_enc", [F, G4], BF16, isOutput=False)
    b_e_d = nc.declare_dram_parameter("b_enc", [1, G4], BF16, isOutput=False)
    whh_d_d = nc.declare_dram_parameter("whhT_dec", [H, G4], BF16, isOutput=False)
    wih_d_d = nc.declare_dram_parameter("wihT_dec", [F, G4], BF16, isOutput=False)
    b_d_d = nc.declare_dram_parameter("b_dec", [1, G4], BF16, isOutput=False)
    wlin_d = nc.declare_dram_parameter("wlinT", [H, F], BF16, isOutput=False)
    ident_d = nc.declare_dram_parameter("ident", [128, 128], F32, isOutput=False)
    identb_d = nc.declare_dram_parameter("identb", [128, 128], BF16, isOutput=False)
    out_d = nc.declare_dram_parameter("out", [BL, t_dec * F], F32, isOutput=True)

    with tile.TileContext(nc) as tc:
        with (
            tc.tile_pool(name="weights", bufs=1) as wpool,
            tc.tile_pool(name="state", bufs=1) as spool,
            tc.tile_pool(name="xslab", bufs=3) as xpool,
            tc.tile_pool(name="sig", bufs=3) as sigpool,
            tc.tile_pool(name="tmp", bufs=4) as tmppool,
            tc.tile_pool(name="hbuf", bufs=3) as hpool,
            tc.tile_pool(name="gps", bufs=4, space=bass.MemorySpace.PSUM) as gpool,
            tc.tile_pool(name="htps", bufs=2, space=bass.MemorySpace.PSUM) as htpool,
            tc.tile_pool(name="wlps", bufs=1, space=bass.MemorySpace.PSUM) as wlpool,
        ):
            # ---- weights into SBUF ----
            whh_e = [wpool.tile([128, G4], BF16, tag=f"whh_e{j}", name=f"whh_e{j}") for j in range(4)]
            whh_dc = [wpool.tile([128, G4], BF16, tag=f"whh_d{j}", name=f"whh_d{j}") for j in range(4)]
            for j in range(4):
                nc.sync.dma_start(whh_e[j][:], whh_e_d[128 * j : 128 * (j + 1), :])
                nc.sync.dma_start(whh_dc[j][:], whh_d_d[128 * j : 128 * (j + 1), :])
            wih_e = wpool.tile([F, G4], BF16, tag="wih_e")
            wih_dc = wpool.tile([F, G4], BF16, tag="wih_d")
            nc.sync.dma_start(wih_e[:], wih_e_d[:])
            nc.sync.dma_start(wih_dc[:], wih_d_d[:])
            b_e = wpool.tile([1, G4], BF16, tag="b_e")
            b_dc = wpool.tile([1, G4], BF16, tag="b_d")
            nc.sync.dma_start(b_e[:], b_e_d[:])
            nc.sync.dma_start(b_dc[:], b_d_d[:])
            wlin = [wpool.tile([128, F], BF16, tag=f"wlin{j}", name=f"wlin{j}") for j in range(4)]
            for j in range(4):
                nc.sync.dma_start(wlin[j][:], wlin_d[128 * j : 128 * (j + 1), :])
            ident = wpool.tile([128, 128], F32, tag="ident")
            nc.sync.dma_start(ident[:], ident_d[:])
            identb = wpool.tile([128, 128], BF16, tag="identb")
            nc.sync.dma_start(identb[:], identb_d[:])
            ones = wpool.tile([1, 128], BF16, tag="ones")
            nc.vector.memset(ones[:], 1.0)

            # ---- state ----
            hT_all = spool.tile([128, 512], BF16, tag="hT_all")
            nc.vector.memset(hT_all[:], 0.0)
            hT = [hT_all[:, 128 * j : 128 * (j + 1)] for j in range(4)]
            c_st = [spool.tile([128, 128], F32, tag=f"c{j}", name=f"c{j}") for j in range(4)]
            for j in range(4):
                nc.vector.memset(c_st[j][:], 0.0)
            out_store = spool.tile([128, t_dec * BL], F32, tag="out_store")

            def lstm_step(x_lhsT, whh, wih, b_t, first):
                """One LSTM cell step. x_lhsT: [F,128] fp32 stationary
                (batch-replicated x_t^T). Updates hT[j], c_st[j] in place."""
                gps = []
                for n in range(NB):
                    g_ps = gpool.tile([128, 512], F32, tag="g_ps")
                    ncol = slice(512 * n, 512 * (n + 1))
                    nc.tensor.matmul(
                        g_ps[:], ones[:], b_t[:, ncol], start=True, stop=False
                    )
                    nc.tensor.matmul(
                        g_ps[:], x_lhsT, wih[:, ncol], start=False, stop=False
                    )
                    for j in range(4):
                        nc.tensor.matmul(
                            g_ps[:],
                            hT[j],
                            whh[j][:, ncol],
                            start=False,
                            stop=(j == 3),
                        )
                    gps.append(g_ps)
                ht_ps = htpool.tile([128, 512], BF16, tag="ht_ps")
                for n in range(NB):
                    g_ps = gps[n]
                    # bank layout: [i(0:128), f(128:256), o(256:384), g(384:512)]
                    sig = sigpool.tile([128, 384], F32, tag="sig")
                    nc.scalar.activation(sig[:], g_ps[:, 0:384], AF.Sigmoid)
                    tg = tmppool.tile([128, 128], F32, tag="tg")
                    nc.scalar.activation(tg[:], g_ps[:, 384:512], AF.Tanh)
                    t1 = tmppool.tile([128, 128], F32, tag="t1")
                    nc.vector.tensor_mul(t1[:], sig[:, 128:256], c_st[n][:])
                    t2 = tmppool.tile([128, 128], F32, tag="t2")
                    nc.vector.tensor_mul(t2[:], sig[:, 0:128], tg[:])
                    nc.vector.tensor_add(c_st[n][:], t1[:], t2[:])
                    tc_t = tmppool.tile([128, 128], F32, tag="tc")
                    nc.scalar.activation(tc_t[:], c_st[n][:], AF.Tanh)
                    h_t = hpool.tile([128, 128], BF16, tag="h")
                    nc.vector.tensor_mul(h_t[:], sig[:, 256:384], tc_t[:])
                    nc.tensor.transpose(
                        ht_ps[:, 128 * n : 128 * (n + 1)], h_t[:], identb[:]
                    )
                nc.vector.tensor_copy(hT_all[:], ht_ps[:])

            # ---- encoder ----
            xsl = None
            for t in range(t_enc):
                if t % SLAB == 0:
                    nsteps = min(SLAB, t_enc - t)
                    xsl = xpool.tile([F, SLAB * 128], BF16, tag="xsl")
                    nc.sync.dma_start(
                        xsl[:, : nsteps * 128],
                        xrep_d[:, t * 128 : (t + nsteps) * 128],
                    )
                k = t % SLAB
                lstm_step(
                    xsl[:, k * 128 : (k + 1) * 128], whh_e, wih_e, b_e, first=(t == 0)
                )

            # ---- decoder ----
            x_last = spool.tile([F, 128], BF16, tag="x_last")
            nc.sync.dma_start(x_last[:], xrep_d[:, (t_enc - 1) * 128 : t_enc * 128])
            dec_in = x_last
            for t in range(t_dec):
                lstm_step(dec_in[:], whh_dc, wih_dc, b_dc, first=False)
                wl_ps = wlpool.tile([128, 128], F32, tag="wl_ps")
                for j in range(4):
                    nc.tensor.matmul(
                        wl_ps[:],
                        wlin[j][:],
                        hT[j],
                        start=(j == 0),
                        stop=(j == 3),
                    )
                outT = hpool.tile([128, 128], BF16, tag="outT")
                nc.vector.tensor_copy(outT[:], wl_ps[:])
                nc.scalar.activation(
                    out_store[:, t * BL : (t + 1) * BL], wl_ps[:, 0:BL], AF.Copy
                )
                dec_in = outT

            # ---- transpose outputs to batch-major and store ----
            out_final = spool.tile([BL, t_dec * F], F32, tag="out_final")
            for t in range(t_dec):
                tr_ps = wlpool.tile([BL, 128], F32, tag="tr_ps")
                nc.tensor.transpose(
                    tr_ps[:], out_store[:, t * BL : (t + 1) * BL], ident[:]
                )
                nc.scalar.activation(
                    out_final[:, t * F : (t + 1) * F], tr_ps[:], AF.Copy
                )
            nc.sync.dma_start(out_d[:], out_final[:])

    nc.compile()
    return nc


def _prep_host(inputs, t_enc=T_IN, t_dec=T_OUT):
    """Host-side transposes/permutations. Returns per-core in_maps."""
    x = np.asarray(inputs["x"], np.float32)
    bsz = x.shape[0]
    bl = bsz // N_CORES

    # chunk-permutation of the 2048 gate rows:
    # bank n holds [i_n, f_n, o_n, g_n] for hidden units 128n:128(n+1)
    # torch gate order in weights: i(0:512) f(512:1024) g(1024:1536) o(1536:2048)
    perm = np.concatenate(
        [
            np.concatenate(
                [
                    np.arange(128 * n, 128 * (n + 1)),  # i
                    512 + np.arange(128 * n, 128 * (n + 1)),  # f
                    1536 + np.arange(128 * n, 128 * (n + 1)),  # o
                    1024 + np.arange(128 * n, 128 * (n + 1)),  # g
                ]
            )
            for n in range(4)
        ]
    )

    def prep_w(wih, whh, b):
        wihT = np.ascontiguousarray(np.asarray(wih, np.float32)[perm].T).astype(ml_dtypes.bfloat16)
        whhT = np.ascontiguousarray(np.asarray(whh, np.float32)[perm].T).astype(ml_dtypes.bfloat16)
        bp = np.ascontiguousarray(np.asarray(b, np.float32)[perm][None, :]).astype(ml_dtypes.bfloat16)
        return wihT, whhT, bp

    wihT_e, whhT_e, b_e = prep_w(inputs["enc_Wih"], inputs["enc_Whh"], inputs["enc_b"])
    wihT_d, whhT_d, b_d = prep_w(inputs["dec_Wih"], inputs["dec_Whh"], inputs["dec_b"])
    wlinT = np.ascontiguousarray(np.asarray(inputs["W_lin"], np.float32).T).astype(ml_dtypes.bfloat16)
    ident = np.eye(128, dtype=np.float32)

    in_maps = []
    for c in range(N_CORES):
        xc = x[c * bl : (c + 1) * bl, :t_enc, :]  # [bl, t_enc, F]
        xt = xc.transpose(2, 1, 0)  # [F, t_enc, bl]
        xrep = np.broadcast_to(
            xt[:, :, None, :], (F, t_enc, 128 // bl, bl)
        ).reshape(F, t_enc * 128)
        in_maps.append(
            {
                "xrep": np.ascontiguousarray(xrep).astype(ml_dtypes.bfloat16),
                "whhT_enc": whhT_e,
                "wihT_enc": wihT_e,
                "b_enc": b_e,
                "whhT_dec": whhT_d,
                "wihT_dec": wihT_d,
                "b_dec": b_d,
                "wlinT": wlinT,
                "ident": ident,
                "identb": ident.astype(ml_dtypes.bfloat16),
            }
        )
    return in_maps


_NC_CACHE = {}


def _get_nc(t_enc, t_dec):
    key = (t_enc, t_dec)
    if key not in _NC_CACHE:
        _NC_CACHE[key] = build_nc(t_enc, t_dec)
    return _NC_CACHE[key]


def kernel(x, enc_Wih, enc_Whh, enc_b, dec_Wih, dec_Whh, dec_b, W_lin, t_out):
    t_out = int(t_out)
    inputs = dict(
        x=x, enc_Wih=enc_Wih, enc_Whh=enc_Whh, enc_b=enc_b,
        dec_Wih=dec_Wih, dec_Whh=dec_Whh, dec_b=dec_b, W_lin=W_lin,
    )
    t_enc = int(np.asarray(x).shape[1])
    nc = _get_nc(t_enc, t_out)
    in_maps = _prep_host(inputs, t_enc, t_out)
    res = run_bass_kernel_spmd(nc, in_maps, list(range(N_CORES)))
    bl = np.asarray(x).shape[0] // N_CORES
    outs = [
        res.results[c]["out"].reshape(bl, t_out, F) for c in range(N_CORES)
    ]
    return np.concatenate(outs, axis=0).astype(np.float32)
